# revision 11
# baseline (speedup 1.0000x reference)
"""Bipolar morphological conv2d kernel for Trainium2 (8 NeuronCores).

Math: reference computes, per output position and out-channel c,
    y = m(lp1,K1) - m(lp1,K2) - m(lp2,K1) + m(lp2,K2) + bias
with m(logp, k)[c] = exp(max_p(logp_p + k_pc)), lp1 = log(max(patch, .1)),
lp2 = log(max(-patch, .1)).

Since exp is monotone, exp(max_p(log(max(x,.1)) + k)) = max_p(max(x,.1)*K_pc)
with K = exp(k) > 0.  Further, the clamp folds into a per-channel constant:
    max_p(max(x_p,.1)*K_pc) = max(U_c, max_p(x_p*K_pc)),  U_c = .1*max_p K_pc
(because entries with x_p <= .1 contribute x_p*K <= .1*K <= U_c, and the true
value is always >= U_c).  Likewise the "-x" side is max(U_c, max_p(-x_p*K_pc)).
So the whole op is 4 max-times reductions over unclamped products x_p*K_pc.

Device strategy (data-parallel, one batch image per core):
  - partitions = 128 = [64 out-channels "A side" (+x) | 64 out-channels "B side" (-x)]
  - free dim   = 900 output positions, addressed as [30 rows, 30 cols] windows
    (row stride 32) into the pixel-linear broadcast row
  - x row per input channel is broadcast across partitions as [+x;...;-x;...]
    via a K=1 PE matmul (lhsT = [+1]*64+[-1]*64) into PSUM, staged to SBUF by
    the Scalar engine.
  - per (tap, ci) one fused scalar_tensor_tensor (mult then max) per kernel
    accumulator: acc_k = max(acc_k, xwin * K_k[(tap,ci), c])  -- 576 DVE ops,
    which is the roofline: DVE f32 3-src ops run at 1 elem/cycle/lane.
  - combine: one accumulating PE matmul pair per 128-position chunk computes
    (accA1-accB1)-(accA2-accB2) transposed to position-major; add bias; DMA.
Host precomputes exp(k), U_c, the packed per-partition scalar tables, and the
transposed/padded x rows.
"""

import os
from contextlib import ExitStack

import numpy as np

import concourse.bass as bass
import concourse.mybir as mybir
from concourse import bacc
import concourse.tile as tile
from concourse.bass_utils import run_bass_kernel_spmd

N_CORES = 8
H = W = C = 32
COUT = 64
HO = WO = 30
NPIX = H * W          # 1024
FD = HO * WO          # 900 output positions, accessed as [30, 30] windows
XLEN = 1026           # broadcast-row length: max tap offset 66 + 30*32 window
P = 288               # 3*3*32 patch size

F32 = mybir.dt.float32
_cache: dict = {}
last_results = None


def _ensure_axon_ntff_hook():
    """The trimmed agent image lacks antenv.axon_hooks; recreate it so
    run_bass_kernel_spmd(trace=True) can capture NTFF profiles. No-op on
    failure (tracing then just degrades)."""
    import sys
    import types

    try:
        import antenv.axon_hooks  # noqa: F401
        return
    except ImportError:
        pass
    try:
        mod = types.ModuleType("antenv.axon_hooks")
        holder = [None]
        mod.set_axon_ntff_profile_hook = lambda h: holder.__setitem__(0, h)
        mod.get_axon_ntff_profile_hook = lambda: holder[0]
        sys.modules["antenv.axon_hooks"] = mod
        from trn_agent_boot.trn_boot import _ntff_profile_via_ctypes

        so = "/opt/axon/libaxon_pjrt.so"
        if os.path.exists(so):
            holder[0] = _ntff_profile_via_ctypes(so)
    except Exception:
        pass


def _build_module():
    nc = bacc.Bacc()
    Alu = mybir.AluOpType

    xT = nc.dram_tensor("xT", [1, C * XLEN], F32, kind="ExternalInput")
    S1 = nc.dram_tensor("S1", [128, P], F32, kind="ExternalInput")
    S2 = nc.dram_tensor("S2", [128, P], F32, kind="ExternalInput")
    UB = nc.dram_tensor("UB", [128, 2], F32, kind="ExternalInput")
    BC = nc.dram_tensor("BC", [128, COUT], F32, kind="ExternalInput")
    PM = nc.dram_tensor("PM", [1, 128], F32, kind="ExternalInput")
    M1 = nc.dram_tensor("M1", [128, COUT], F32, kind="ExternalInput")
    M2 = nc.dram_tensor("M2", [128, COUT], F32, kind="ExternalInput")
    Y = nc.dram_tensor("Y", [HO * WO, COUT], F32, kind="ExternalOutput")

    with tile.TileContext(nc) as tc, ExitStack() as ctx:
        const = ctx.enter_context(tc.tile_pool(name="const", bufs=1))
        xbp = ctx.enter_context(tc.tile_pool(name="xbp", bufs=2, space="PSUM"))
        xbs = ctx.enter_context(tc.tile_pool(name="xbs", bufs=3))
        accp = ctx.enter_context(tc.tile_pool(name="accp", bufs=1))
        tps = ctx.enter_context(tc.tile_pool(name="tps", bufs=2, space="PSUM"))
        tsb = ctx.enter_context(tc.tile_pool(name="tsb", bufs=2))

        xT_sb = const.tile([1, C * XLEN], F32)
        nc.gpsimd.dma_start(out=xT_sb[:, :], in_=xT[:, :])
        S1_sb = const.tile([128, P], F32)
        nc.gpsimd.dma_start(out=S1_sb[:, :], in_=S1[:, :])
        S2_sb = const.tile([128, P], F32)
        nc.gpsimd.dma_start(out=S2_sb[:, :], in_=S2[:, :])
        UB_sb = const.tile([128, 2], F32)
        nc.gpsimd.dma_start(out=UB_sb[:, :], in_=UB[:, :])
        BC_sb = const.tile([128, COUT], F32)
        nc.gpsimd.dma_start(out=BC_sb[:, :], in_=BC[:, :])
        PM_sb = const.tile([1, 128], F32)
        nc.gpsimd.dma_start(out=PM_sb[:, :], in_=PM[:, :])
        M1_sb = const.tile([128, COUT], F32)
        nc.gpsimd.dma_start(out=M1_sb[:, :], in_=M1[:, :])
        M2_sb = const.tile([128, COUT], F32)
        nc.gpsimd.dma_start(out=M2_sb[:, :], in_=M2[:, :])

        # acc_k[p<64]  = running max(U_c, max(x*Kk)),  p in [64,128): -x side
        acc1 = accp.tile([128, FD], F32)
        acc2 = accp.tile([128, FD], F32)
        acc1w = acc1[:, :].rearrange("q (a b) -> q a b", a=HO)
        acc2w = acc2[:, :].rearrange("q (a b) -> q a b", a=HO)
        nc.vector.memset(acc1[:, :], 0.0)
        nc.vector.tensor_scalar(
            out=acc1[:, :], in0=acc1[:, :], scalar1=UB_sb[:, 0:1], scalar2=None,
            op0=Alu.add,
        )
        nc.vector.memset(acc2[:, :], 0.0)
        nc.vector.tensor_scalar(
            out=acc2[:, :], in0=acc2[:, :], scalar1=UB_sb[:, 1:2], scalar2=None,
            op0=Alu.add,
        )

        for ci in range(C):
            # broadcast row ci of xT to [ +x (64 parts) ; -x (64 parts) ]
            xq = xbp.tile([128, XLEN], F32)
            for s, e in ((0, 512), (512, 1024), (1024, XLEN)):
                nc.tensor.matmul(
                    xq[:, s:e], lhsT=PM_sb[:, :], rhs=xT_sb[0:1, ci * XLEN + s : ci * XLEN + e],
                    start=True, stop=True,
                )
            xb = xbs.tile([128, XLEN], F32)
            nc.scalar.copy(out=xb[:, :], in_=xq[:, :])

            for t in range(9):
                i, j = divmod(t, 3)
                off = i * W + j
                p = t * C + ci
                # 30x30 output window at tap offset, row stride W
                in0 = xb[:, off : off + HO * W].rearrange(
                    "q (a b) -> q a b", b=W)[:, :, :WO]
                nc.vector.scalar_tensor_tensor(
                    out=acc1w, in0=in0, scalar=S1_sb[:, p : p + 1],
                    in1=acc1w, op0=Alu.mult, op1=Alu.max,
                )
                nc.vector.scalar_tensor_tensor(
                    out=acc2w, in0=in0, scalar=S2_sb[:, p : p + 1],
                    in1=acc2w, op0=Alu.mult, op1=Alu.max,
                )

        # Combine + transpose in one PE op per 128-pos chunk:
        #   pt = acc1_chunk.T @ [I;-I]  +  acc2_chunk.T @ [-I;I]
        #      = (accA1-accB1) - (accA2-accB2), position-major [cw, 64].
        # Then add the partition-replicated bias and DMA the chunk out.
        for c0 in range(0, FD, 128):
            cw = min(128, FD - c0)
            pt = tps.tile([128, COUT], F32)
            nc.tensor.matmul(pt[:cw, :], lhsT=acc1[:, c0 : c0 + cw], rhs=M1_sb[:, :],
                             start=True, stop=False)
            nc.tensor.matmul(pt[:cw, :], lhsT=acc2[:, c0 : c0 + cw], rhs=M2_sb[:, :],
                             start=False, stop=True)
            ysb = tsb.tile([128, COUT], F32)
            nc.vector.tensor_tensor(ysb[:cw, :], pt[:cw, :], BC_sb[:cw, :], Alu.add)
            nc.sync.dma_start(out=Y[c0 : c0 + cw, :], in_=ysb[:cw, :])
    nc.finalize()
    return nc


def _host_prep(x, k1, k2, bias):
    x = np.ascontiguousarray(np.asarray(x, dtype=np.float32))
    K1 = np.exp(np.asarray(k1, np.float32).reshape(P, COUT))
    K2 = np.exp(np.asarray(k2, np.float32).reshape(P, COUT))
    S1 = np.vstack([K1.T, K1.T]).astype(np.float32)          # [128, 288]
    S2 = np.vstack([K2.T, K2.T]).astype(np.float32)
    U1 = 0.1 * K1.max(axis=0)
    U2 = 0.1 * K2.max(axis=0)
    UB = np.stack([np.concatenate([U1, U1]), np.concatenate([U2, U2])], axis=1)
    UB = np.ascontiguousarray(UB, np.float32)                # [128, 2]
    BC = np.tile(np.asarray(bias, np.float32).reshape(1, COUT), (128, 1))
    PM = np.concatenate([np.ones(64, np.float32), -np.ones(64, np.float32)]).reshape(1, 128)
    M1 = np.vstack([np.eye(COUT, dtype=np.float32), -np.eye(COUT, dtype=np.float32)])
    M2 = np.ascontiguousarray(-M1)
    shared = dict(S1=S1, S2=S2, UB=UB, BC=np.ascontiguousarray(BC),
                  PM=np.ascontiguousarray(PM), M1=np.ascontiguousarray(M1), M2=M2)
    in_maps = []
    for n in range(N_CORES):
        xT = np.zeros((C, XLEN), np.float32)
        xT[:, :NPIX] = x[n].reshape(NPIX, C).T
        in_maps.append({"xT": xT.reshape(1, C * XLEN), **shared})
    return in_maps


def kernel(x, k1, k2, bias):
    global last_results
    if "nc" not in _cache:
        _cache["nc"] = _build_module()
    nc = _cache["nc"]
    in_maps = _host_prep(x, k1, k2, bias)
    trace = bool(int(os.environ.get("KTRACE", "0")))
    if trace:
        _ensure_axon_ntff_hook()
    res = run_bass_kernel_spmd(
        nc, in_maps, core_ids=list(range(N_CORES)), trace=trace,
    )
    last_results = res
    y = np.stack([r["Y"].reshape(HO, WO, COUT) for r in res.results], axis=0)
    return y.astype(np.float32)


# revision 12
# speedup vs baseline: 1.2069x; 1.2069x over previous
"""Bipolar morphological conv2d kernel for Trainium2 (8 NeuronCores).

Math: reference computes, per output position and out-channel c,
    y = m(lp1,K1) - m(lp1,K2) - m(lp2,K1) + m(lp2,K2) + bias
with m(logp, k)[c] = exp(max_p(logp_p + k_pc)), lp1 = log(max(patch, .1)),
lp2 = log(max(-patch, .1)).

Since exp is monotone, exp(max_p(log(max(x,.1)) + k)) = max_p(max(x,.1)*K_pc)
with K = exp(k) > 0.  Further, the clamp folds into a per-channel constant:
    max_p(max(x_p,.1)*K_pc) = max(U_c, max_p(x_p*K_pc)),  U_c = .1*max_p K_pc
(because entries with x_p <= .1 contribute x_p*K <= .1*K <= U_c, and the true
value is always >= U_c).  Likewise the "-x" side is max(U_c, max_p(-x_p*K_pc)).
So the whole op is 4 max-times reductions over unclamped products x_p*K_pc.

Device strategy (data-parallel, one batch image per core):
  - partitions = 128 = [64 out-channels "A side" (+x) | 64 out-channels "B side" (-x)]
  - free dim   = 900 output positions, addressed as [30 rows, 30 cols] windows
    (row stride 32) into the pixel-linear broadcast row
  - x row per input channel is broadcast across partitions as [+x;...;-x;...]
    via a K=1 PE matmul (lhsT = [+1]*64+[-1]*64) into PSUM, staged to SBUF by
    the Scalar engine.
  - per (tap, ci) one fused scalar_tensor_tensor (mult then max) per kernel
    accumulator: acc_k = max(acc_k, xwin * K_k[(tap,ci), c])  -- 576 DVE ops,
    which is the roofline: DVE f32 3-src ops run at 1 elem/cycle/lane.
  - combine: one accumulating PE matmul pair per 128-position chunk computes
    (accA1-accB1)-(accA2-accB2) transposed to position-major; add bias; DMA.
Host precomputes exp(k), U_c, the packed per-partition scalar tables, and the
transposed/padded x rows.
"""

import os
from contextlib import ExitStack

import numpy as np

import concourse.bass as bass
import concourse.mybir as mybir
from concourse import bacc
import concourse.tile as tile
from concourse.bass_utils import run_bass_kernel_spmd

N_CORES = 8
H = W = C = 32
COUT = 64
HO = WO = 30
NPIX = H * W          # 1024
FD = HO * WO          # 900 output positions, accessed as [30, 30] windows
XLEN = 1026           # broadcast-row length: max tap offset 66 + 30*32 window
P = 288               # 3*3*32 patch size

F32 = mybir.dt.float32
F16 = mybir.dt.float16
_cache: dict = {}
last_results = None


def _ensure_axon_ntff_hook():
    """The trimmed agent image lacks antenv.axon_hooks; recreate it so
    run_bass_kernel_spmd(trace=True) can capture NTFF profiles. No-op on
    failure (tracing then just degrades)."""
    import sys
    import types

    try:
        import antenv.axon_hooks  # noqa: F401
        return
    except ImportError:
        pass
    try:
        mod = types.ModuleType("antenv.axon_hooks")
        holder = [None]
        mod.set_axon_ntff_profile_hook = lambda h: holder.__setitem__(0, h)
        mod.get_axon_ntff_profile_hook = lambda: holder[0]
        sys.modules["antenv.axon_hooks"] = mod
        from trn_agent_boot.trn_boot import _ntff_profile_via_ctypes

        so = "/opt/axon/libaxon_pjrt.so"
        if os.path.exists(so):
            holder[0] = _ntff_profile_via_ctypes(so)
    except Exception:
        pass


def _build_module():
    nc = bacc.Bacc()
    Alu = mybir.AluOpType

    xT = nc.dram_tensor("xT", [1, C * XLEN], F32, kind="ExternalInput")
    S1 = nc.dram_tensor("S1", [128, P], F32, kind="ExternalInput")
    S2 = nc.dram_tensor("S2", [128, P], F32, kind="ExternalInput")
    UB = nc.dram_tensor("UB", [128, 2], F32, kind="ExternalInput")
    BC = nc.dram_tensor("BC", [128, COUT], F32, kind="ExternalInput")
    PM = nc.dram_tensor("PM", [1, 128], F32, kind="ExternalInput")
    M1 = nc.dram_tensor("M1", [128, COUT], F16, kind="ExternalInput")
    M2 = nc.dram_tensor("M2", [128, COUT], F16, kind="ExternalInput")
    Y = nc.dram_tensor("Y", [HO * WO, COUT], F32, kind="ExternalOutput")

    with tile.TileContext(nc) as tc, ExitStack() as ctx:
        const = ctx.enter_context(tc.tile_pool(name="const", bufs=1))
        xbp = ctx.enter_context(tc.tile_pool(name="xbp", bufs=2, space="PSUM"))
        xbs = ctx.enter_context(tc.tile_pool(name="xbs", bufs=3))
        accp = ctx.enter_context(tc.tile_pool(name="accp", bufs=1))
        prodp = ctx.enter_context(tc.tile_pool(name="prodp", bufs=2))
        tps = ctx.enter_context(tc.tile_pool(name="tps", bufs=2, space="PSUM"))
        tsb = ctx.enter_context(tc.tile_pool(name="tsb", bufs=2))

        xT_sb = const.tile([1, C * XLEN], F32)
        nc.gpsimd.dma_start(out=xT_sb[:, :], in_=xT[:, :])
        S1_sb = const.tile([128, P], F32)
        nc.gpsimd.dma_start(out=S1_sb[:, :], in_=S1[:, :])
        S2_sb = const.tile([128, P], F32)
        nc.gpsimd.dma_start(out=S2_sb[:, :], in_=S2[:, :])
        UB_sb = const.tile([128, 2], F32)
        nc.gpsimd.dma_start(out=UB_sb[:, :], in_=UB[:, :])
        BC_sb = const.tile([128, COUT], F32)
        nc.gpsimd.dma_start(out=BC_sb[:, :], in_=BC[:, :])
        PM_sb = const.tile([1, 128], F32)
        nc.gpsimd.dma_start(out=PM_sb[:, :], in_=PM[:, :])
        M1_sb = const.tile([128, COUT], F16)
        nc.gpsimd.dma_start(out=M1_sb[:, :], in_=M1[:, :])
        M2_sb = const.tile([128, COUT], F16)
        nc.gpsimd.dma_start(out=M2_sb[:, :], in_=M2[:, :])

        # acc12 = [K1 | K2] accs side by side, fp16; partitions = [A(+x)|B(-x)]
        acc12 = accp.tile([128, 2 * FD], F16)
        nc.vector.memset(acc12[:, :], 0.0)
        nc.vector.tensor_scalar(
            out=acc12[:, 0:FD], in0=acc12[:, 0:FD], scalar1=UB_sb[:, 0:1],
            scalar2=None, op0=Alu.add,
        )
        nc.vector.tensor_scalar(
            out=acc12[:, FD : 2 * FD], in0=acc12[:, FD : 2 * FD],
            scalar1=UB_sb[:, 1:2], scalar2=None, op0=Alu.add,
        )
        p1w = None

        for ci in range(C):
            # broadcast row ci of xT to [ +x (64 parts) ; -x (64 parts) ]
            xq = xbp.tile([128, XLEN], F32)
            for s, e in ((0, 512), (512, 1024), (1024, XLEN)):
                nc.tensor.matmul(
                    xq[:, s:e], lhsT=PM_sb[:, :], rhs=xT_sb[0:1, ci * XLEN + s : ci * XLEN + e],
                    start=True, stop=True,
                )
            # fp16 staging, two parities so every tap window is 4B-aligned
            xbE = xbs.tile([128, XLEN], F16, tag="xbE")
            nc.scalar.copy(out=xbE[:, :], in_=xq[:, :])
            xbO = xbs.tile([128, XLEN - 1], F16, tag="xbO")
            nc.scalar.copy(out=xbO[:, :], in_=xq[:, 1:XLEN])

            for t in range(9):
                i, j = divmod(t, 3)
                off = i * W + j
                p = t * C + ci
                # 30x30 output window at tap offset, row stride W (even base)
                if off % 2 == 0:
                    src = xbE[:, off : off + HO * W]
                else:
                    src = xbO[:, off - 1 : off - 1 + HO * W]
                in0 = src.rearrange("q (a b) -> q a b", b=W)[:, :, :WO]
                prod = prodp.tile([128, 2 * FD], F16)
                nc.vector.tensor_scalar(
                    out=prod[:, 0:FD].rearrange("q (a b) -> q a b", a=HO),
                    in0=in0, scalar1=S1_sb[:, p : p + 1], scalar2=None,
                    op0=Alu.mult,
                )
                nc.vector.tensor_scalar(
                    out=prod[:, FD : 2 * FD].rearrange("q (a b) -> q a b", a=HO),
                    in0=in0, scalar1=S2_sb[:, p : p + 1], scalar2=None,
                    op0=Alu.mult,
                )
                nc.vector.tensor_tensor(
                    acc12[:, :], prod[:, :], acc12[:, :], Alu.max,
                )

        # Combine + transpose in one PE op per 128-pos chunk:
        #   pt = acc1_chunk.T @ [I;-I]  +  acc2_chunk.T @ [-I;I]
        #      = (accA1-accB1) - (accA2-accB2), position-major [cw, 64].
        # Then add the partition-replicated bias and DMA the chunk out.
        for c0 in range(0, FD, 128):
            cw = min(128, FD - c0)
            pt = tps.tile([128, COUT], F32)
            nc.tensor.matmul(pt[:cw, :], lhsT=acc12[:, c0 : c0 + cw], rhs=M1_sb[:, :],
                             start=True, stop=False)
            nc.tensor.matmul(pt[:cw, :], lhsT=acc12[:, FD + c0 : FD + c0 + cw], rhs=M2_sb[:, :],
                             start=False, stop=True)
            ysb = tsb.tile([128, COUT], F32)
            nc.vector.tensor_tensor(ysb[:cw, :], pt[:cw, :], BC_sb[:cw, :], Alu.add)
            nc.sync.dma_start(out=Y[c0 : c0 + cw, :], in_=ysb[:cw, :])
    nc.finalize()
    return nc


def _host_prep(x, k1, k2, bias):
    x = np.ascontiguousarray(np.asarray(x, dtype=np.float32))
    K1 = np.exp(np.asarray(k1, np.float32).reshape(P, COUT))
    K2 = np.exp(np.asarray(k2, np.float32).reshape(P, COUT))
    S1 = np.vstack([K1.T, K1.T]).astype(np.float32)          # [128, 288]
    S2 = np.vstack([K2.T, K2.T]).astype(np.float32)
    U1 = 0.1 * K1.max(axis=0)
    U2 = 0.1 * K2.max(axis=0)
    UB = np.stack([np.concatenate([U1, U1]), np.concatenate([U2, U2])], axis=1)
    UB = np.ascontiguousarray(UB, np.float32)                # [128, 2]
    BC = np.tile(np.asarray(bias, np.float32).reshape(1, COUT), (128, 1))
    PM = np.concatenate([np.ones(64, np.float32), -np.ones(64, np.float32)]).reshape(1, 128)
    M1 = np.vstack([np.eye(COUT, dtype=np.float16), -np.eye(COUT, dtype=np.float16)])
    M2 = np.ascontiguousarray(-M1)
    shared = dict(S1=S1, S2=S2, UB=UB, BC=np.ascontiguousarray(BC),
                  PM=np.ascontiguousarray(PM), M1=np.ascontiguousarray(M1), M2=M2)
    in_maps = []
    for n in range(N_CORES):
        xT = np.zeros((C, XLEN), np.float32)
        xT[:, :NPIX] = x[n].reshape(NPIX, C).T
        in_maps.append({"xT": xT.reshape(1, C * XLEN), **shared})
    return in_maps


def kernel(x, k1, k2, bias):
    global last_results
    if "nc" not in _cache:
        _cache["nc"] = _build_module()
    nc = _cache["nc"]
    in_maps = _host_prep(x, k1, k2, bias)
    trace = bool(int(os.environ.get("KTRACE", "0")))
    if trace:
        _ensure_axon_ntff_hook()
    res = run_bass_kernel_spmd(
        nc, in_maps, core_ids=list(range(N_CORES)), trace=trace,
    )
    last_results = res
    y = np.stack([r["Y"].reshape(HO, WO, COUT) for r in res.results], axis=0)
    return y.astype(np.float32)


# revision 13
# speedup vs baseline: 1.2083x; 1.0011x over previous
"""Bipolar morphological conv2d kernel for Trainium2 (8 NeuronCores).

Math: reference computes, per output position and out-channel c,
    y = m(lp1,K1) - m(lp1,K2) - m(lp2,K1) + m(lp2,K2) + bias
with m(logp, k)[c] = exp(max_p(logp_p + k_pc)), lp1 = log(max(patch, .1)),
lp2 = log(max(-patch, .1)).

Since exp is monotone, exp(max_p(log(max(x,.1)) + k)) = max_p(max(x,.1)*K_pc)
with K = exp(k) > 0.  Further, the clamp folds into a per-channel constant:
    max_p(max(x_p,.1)*K_pc) = max(U_c, max_p(x_p*K_pc)),  U_c = .1*max_p K_pc
(because entries with x_p <= .1 contribute x_p*K <= .1*K <= U_c, and the true
value is always >= U_c).  Likewise the "-x" side is max(U_c, max_p(-x_p*K_pc)).
So the whole op is 4 max-times reductions over unclamped products x_p*K_pc.

Device strategy (data-parallel, one batch image per core):
  - partitions = 128 = [64 out-channels "A side" (+x) | 64 out-channels "B side" (-x)]
  - free dim   = 900 output positions, addressed as [30 rows, 30 cols] windows
    (row stride 32) into the pixel-linear broadcast row
  - x row per input channel is broadcast across partitions as [+x;...;-x;...]
    via a K=1 PE matmul (lhsT = [+1]*64+[-1]*64) into PSUM, staged to SBUF by
    the Scalar engine.
  - per (tap, ci) one fused scalar_tensor_tensor (mult then max) per kernel
    accumulator: acc_k = max(acc_k, xwin * K_k[(tap,ci), c])  -- 576 DVE ops,
    which is the roofline: DVE f32 3-src ops run at 1 elem/cycle/lane.
  - combine: one accumulating PE matmul pair per 128-position chunk computes
    (accA1-accB1)-(accA2-accB2) transposed to position-major; add bias; DMA.
Host precomputes exp(k), U_c, the packed per-partition scalar tables, and the
transposed/padded x rows.
"""

import os
from contextlib import ExitStack

import numpy as np

import concourse.bass as bass
import concourse.mybir as mybir
from concourse import bacc
import concourse.tile as tile
from concourse.bass_utils import run_bass_kernel_spmd

N_CORES = 8
H = W = C = 32
COUT = 64
HO = WO = 30
NPIX = H * W          # 1024
FD = HO * WO          # 900 output positions, accessed as [30, 30] windows
XLEN = 1026           # broadcast-row length: max tap offset 66 + 30*32 window
P = 288               # 3*3*32 patch size

F32 = mybir.dt.float32
F16 = mybir.dt.float16
_cache: dict = {}
last_results = None


def _ensure_axon_ntff_hook():
    """The trimmed agent image lacks antenv.axon_hooks; recreate it so
    run_bass_kernel_spmd(trace=True) can capture NTFF profiles. No-op on
    failure (tracing then just degrades)."""
    import sys
    import types

    try:
        import antenv.axon_hooks  # noqa: F401
        return
    except ImportError:
        pass
    try:
        mod = types.ModuleType("antenv.axon_hooks")
        holder = [None]
        mod.set_axon_ntff_profile_hook = lambda h: holder.__setitem__(0, h)
        mod.get_axon_ntff_profile_hook = lambda: holder[0]
        sys.modules["antenv.axon_hooks"] = mod
        from trn_agent_boot.trn_boot import _ntff_profile_via_ctypes

        so = "/opt/axon/libaxon_pjrt.so"
        if os.path.exists(so):
            holder[0] = _ntff_profile_via_ctypes(so)
    except Exception:
        pass


def _build_module():
    nc = bacc.Bacc()
    Alu = mybir.AluOpType

    xT = nc.dram_tensor("xT", [1, C * XLEN], F32, kind="ExternalInput")
    S1 = nc.dram_tensor("S1", [128, P], F32, kind="ExternalInput")
    S2 = nc.dram_tensor("S2", [128, P], F32, kind="ExternalInput")
    UB = nc.dram_tensor("UB", [128, 2], F32, kind="ExternalInput")
    BC = nc.dram_tensor("BC", [128, COUT], F32, kind="ExternalInput")
    PM = nc.dram_tensor("PM", [1, 128], F32, kind="ExternalInput")
    M1 = nc.dram_tensor("M1", [128, COUT], F16, kind="ExternalInput")
    M2 = nc.dram_tensor("M2", [128, COUT], F16, kind="ExternalInput")
    Y = nc.dram_tensor("Y", [HO * WO, COUT], F32, kind="ExternalOutput")

    with tile.TileContext(nc) as tc, ExitStack() as ctx:
        const = ctx.enter_context(tc.tile_pool(name="const", bufs=1))
        xbp = ctx.enter_context(tc.tile_pool(name="xbp", bufs=2, space="PSUM"))
        xbs = ctx.enter_context(tc.tile_pool(name="xbs", bufs=3))
        accp = ctx.enter_context(tc.tile_pool(name="accp", bufs=1))
        prodp = ctx.enter_context(tc.tile_pool(name="prodp", bufs=3))
        tps = ctx.enter_context(tc.tile_pool(name="tps", bufs=2, space="PSUM"))
        tsb = ctx.enter_context(tc.tile_pool(name="tsb", bufs=2))

        xT_sb = const.tile([1, C * XLEN], F32)
        nc.gpsimd.dma_start(out=xT_sb[:, :], in_=xT[:, :])
        S1_sb = const.tile([128, P], F32)
        nc.gpsimd.dma_start(out=S1_sb[:, :], in_=S1[:, :])
        S2_sb = const.tile([128, P], F32)
        nc.gpsimd.dma_start(out=S2_sb[:, :], in_=S2[:, :])
        UB_sb = const.tile([128, 2], F32)
        nc.gpsimd.dma_start(out=UB_sb[:, :], in_=UB[:, :])
        BC_sb = const.tile([128, COUT], F32)
        nc.gpsimd.dma_start(out=BC_sb[:, :], in_=BC[:, :])
        PM_sb = const.tile([1, 128], F32)
        nc.gpsimd.dma_start(out=PM_sb[:, :], in_=PM[:, :])
        M1_sb = const.tile([128, COUT], F16)
        nc.gpsimd.dma_start(out=M1_sb[:, :], in_=M1[:, :])
        M2_sb = const.tile([128, COUT], F16)
        nc.gpsimd.dma_start(out=M2_sb[:, :], in_=M2[:, :])

        # acc12 = [K1 | K2] accs side by side, fp16; partitions = [A(+x)|B(-x)]
        acc12 = accp.tile([128, 2 * FD], F16)
        nc.vector.memset(acc12[:, :], 0.0)
        nc.vector.tensor_scalar(
            out=acc12[:, 0:FD], in0=acc12[:, 0:FD], scalar1=UB_sb[:, 0:1],
            scalar2=None, op0=Alu.add,
        )
        nc.vector.tensor_scalar(
            out=acc12[:, FD : 2 * FD], in0=acc12[:, FD : 2 * FD],
            scalar1=UB_sb[:, 1:2], scalar2=None, op0=Alu.add,
        )
        pending = None  # software pipeline: fold products one iter late

        for ci in range(C):
            # broadcast row ci of xT to [ +x (64 parts) ; -x (64 parts) ]
            xq = xbp.tile([128, XLEN], F32)
            for s, e in ((0, 512), (512, 1024), (1024, XLEN)):
                nc.tensor.matmul(
                    xq[:, s:e], lhsT=PM_sb[:, :], rhs=xT_sb[0:1, ci * XLEN + s : ci * XLEN + e],
                    start=True, stop=True,
                )
            # fp16 staging, two parities so every tap window is 4B-aligned
            xbE = xbs.tile([128, XLEN], F16, tag="xbE")
            nc.scalar.copy(out=xbE[:, :], in_=xq[:, :])
            xbO = xbs.tile([128, XLEN - 1], F16, tag="xbO")
            nc.scalar.copy(out=xbO[:, :], in_=xq[:, 1:XLEN])

            for t in range(9):
                i, j = divmod(t, 3)
                off = i * W + j
                p = t * C + ci
                # 30x30 output window at tap offset, row stride W (even base)
                if off % 2 == 0:
                    src = xbE[:, off : off + HO * W]
                else:
                    src = xbO[:, off - 1 : off - 1 + HO * W]
                in0 = src.rearrange("q (a b) -> q a b", b=W)[:, :, :WO]
                prod = prodp.tile([128, 2 * FD], F16)
                nc.vector.tensor_scalar(
                    out=prod[:, 0:FD].rearrange("q (a b) -> q a b", a=HO),
                    in0=in0, scalar1=S1_sb[:, p : p + 1], scalar2=None,
                    op0=Alu.mult,
                )
                nc.vector.tensor_scalar(
                    out=prod[:, FD : 2 * FD].rearrange("q (a b) -> q a b", a=HO),
                    in0=in0, scalar1=S2_sb[:, p : p + 1], scalar2=None,
                    op0=Alu.mult,
                )
                if pending is not None:
                    nc.vector.tensor_tensor(
                        acc12[:, :], pending[:, :], acc12[:, :], Alu.max,
                    )
                pending = prod

        nc.vector.tensor_tensor(
            acc12[:, :], pending[:, :], acc12[:, :], Alu.max,
        )

        # Combine + transpose in one PE op per 128-pos chunk:
        #   pt = acc1_chunk.T @ [I;-I]  +  acc2_chunk.T @ [-I;I]
        #      = (accA1-accB1) - (accA2-accB2), position-major [cw, 64].
        # Then add the partition-replicated bias and DMA the chunk out.
        for c0 in range(0, FD, 128):
            cw = min(128, FD - c0)
            pt = tps.tile([128, COUT], F32)
            nc.tensor.matmul(pt[:cw, :], lhsT=acc12[:, c0 : c0 + cw], rhs=M1_sb[:, :],
                             start=True, stop=False)
            nc.tensor.matmul(pt[:cw, :], lhsT=acc12[:, FD + c0 : FD + c0 + cw], rhs=M2_sb[:, :],
                             start=False, stop=True)
            ysb = tsb.tile([128, COUT], F32)
            nc.vector.tensor_tensor(ysb[:cw, :], pt[:cw, :], BC_sb[:cw, :], Alu.add)
            nc.sync.dma_start(out=Y[c0 : c0 + cw, :], in_=ysb[:cw, :])
    nc.finalize()
    return nc


def _host_prep(x, k1, k2, bias):
    x = np.ascontiguousarray(np.asarray(x, dtype=np.float32))
    K1 = np.exp(np.asarray(k1, np.float32).reshape(P, COUT))
    K2 = np.exp(np.asarray(k2, np.float32).reshape(P, COUT))
    S1 = np.vstack([K1.T, K1.T]).astype(np.float32)          # [128, 288]
    S2 = np.vstack([K2.T, K2.T]).astype(np.float32)
    U1 = 0.1 * K1.max(axis=0)
    U2 = 0.1 * K2.max(axis=0)
    UB = np.stack([np.concatenate([U1, U1]), np.concatenate([U2, U2])], axis=1)
    UB = np.ascontiguousarray(UB, np.float32)                # [128, 2]
    BC = np.tile(np.asarray(bias, np.float32).reshape(1, COUT), (128, 1))
    PM = np.concatenate([np.ones(64, np.float32), -np.ones(64, np.float32)]).reshape(1, 128)
    M1 = np.vstack([np.eye(COUT, dtype=np.float16), -np.eye(COUT, dtype=np.float16)])
    M2 = np.ascontiguousarray(-M1)
    shared = dict(S1=S1, S2=S2, UB=UB, BC=np.ascontiguousarray(BC),
                  PM=np.ascontiguousarray(PM), M1=np.ascontiguousarray(M1), M2=M2)
    in_maps = []
    for n in range(N_CORES):
        xT = np.zeros((C, XLEN), np.float32)
        xT[:, :NPIX] = x[n].reshape(NPIX, C).T
        in_maps.append({"xT": xT.reshape(1, C * XLEN), **shared})
    return in_maps


def kernel(x, k1, k2, bias):
    global last_results
    if "nc" not in _cache:
        _cache["nc"] = _build_module()
    nc = _cache["nc"]
    in_maps = _host_prep(x, k1, k2, bias)
    trace = bool(int(os.environ.get("KTRACE", "0")))
    if trace:
        _ensure_axon_ntff_hook()
    res = run_bass_kernel_spmd(
        nc, in_maps, core_ids=list(range(N_CORES)), trace=trace,
    )
    last_results = res
    y = np.stack([r["Y"].reshape(HO, WO, COUT) for r in res.results], axis=0)
    return y.astype(np.float32)
